# revision 33
# baseline (speedup 1.0000x reference)
"""Trainium2 Bass kernel for the SD-style spatial attention block:

    y = x + out_w @ attn(qkv(groupnorm(x))) + out_b    (per sample)

x: [4, 256, 64, 64] fp32.  GroupNorm(8 groups) -> 1x1 conv QKV (4 heads,
head_dim 32, seq = 64*64 = 4096) -> softmax attention -> 1x1 out conv + bias
+ residual.

Sharding over 8 NeuronCores: core c handles batch b = c//2 and query-half
h = c%2 (2048 of the 4096 query positions).  Each core receives the full
sample (for GroupNorm stats and K/V over all positions) plus its query
slice, and produces the disjoint output slice y[b][:, 2048*h : 2048*(h+1)].
The host concatenates the 8 slices -- no cross-core reduction.

v10 pipeline (per core), designed to make ScalarE (exp) the only
bottleneck and keep it gapless:
  - attention runs in 256 half-slots (chunk c of 512 queries, j-tile t of
    128 keys, head-pair p in {01, 23}).  S^T half-tiles [128, 1024] live
    in a double-buffered 2-bank PSUM pool, so the next half-slot's S
    matmuls (PE) overlap the current exp (ScalarE).
  - softmax denominators come from DVE fp16 adds (4x perf mode) of the
    exp output A into a per-chunk [128, 2048] fp16 accumulator, then one
    ones-matmul per head per chunk - the per-slot ones-matmuls that used
    to eat a third of the PE are gone.
  - exp is computed as exp(S*scale - 2); the constant shift keeps A and
    the fp16 partial sums in fp16 range and cancels in O/D.
  - QKV/out projections consume x directly as f32r (no bf16 cast pass).
"""
import sys

sys.path.insert(0, "/opt/trn_rl_repo")

import numpy as np

import concourse.bass as bass
import concourse.bacc as bacc
import concourse.tile as tile
from concourse import mybir
from concourse.bass_utils import run_bass_kernel_spmd

F32 = mybir.dt.float32
F32R = mybir.dt.float32r
BF16 = mybir.dt.bfloat16
FP16 = mybir.dt.float16
AF = mybir.ActivationFunctionType
OP = mybir.AluOpType

C = 256          # input channels
HID = 128        # qkv hidden (4 heads x 32)
NH = 4
HD = 32
SEQ = 4096       # 64*64 spatial positions
HALF = 2048      # query positions per core
G = 8            # groups
EPS = 1e-5
SCALE = float(HD) ** -0.5
ESHIFT = -2.0    # constant exp shift; cancels in O/D normalization

N_IC = HALF // 512   # i-chunks per core (4)
N_JT = SEQ // 128    # j-tiles (32)


def build_program():
    nc = bacc.Bacc()

    x_kv = nc.declare_dram_parameter("x_kv", [C, SEQ], BF16, isOutput=False)
    x_q = nc.declare_dram_parameter("x_q", [C, HALF], F32R, isOutput=False)
    wqkvT = nc.declare_dram_parameter("wqkvT", [C, 3 * HID], F32, isOutput=False)
    owT = nc.declare_dram_parameter("owT", [HID, C], F32, isOutput=False)
    nw = nc.declare_dram_parameter("nw", [C, 1], F32, isOutput=False)
    nb = nc.declare_dram_parameter("nb", [C, 1], F32, isOutput=False)
    ob = nc.declare_dram_parameter("ob", [C, 1], F32, isOutput=False)
    gsel = nc.declare_dram_parameter("gsel", [C, 128], F32, isOutput=False)
    gselT = nc.declare_dram_parameter("gselT", [128, C], F32, isOutput=False)
    bsel = nc.declare_dram_parameter("bsel", [128, 128], F32, isOutput=False)
    ident = nc.declare_dram_parameter("ident", [128, 128], FP16, isOutput=False)
    y = nc.declare_dram_parameter("y", [C, HALF], F32, isOutput=True)

    with tile.TileContext(nc) as tc:
        import contextlib
        with contextlib.ExitStack() as ctx:
            persist = ctx.enter_context(tc.tile_pool(name="persist", bufs=1))

            # ---------------- load persistent tensors ----------------
            # weights staged in fp32, laundered to f32r via DVE copies
            wq_s = [persist.tile([128, 3 * HID], F32, tag=f"wqs{i}", name=f"wqs{i}") for i in range(2)]
            w_r = [persist.tile([128, 3 * HID], F32R, tag=f"wqr{i}", name=f"wqr{i}") for i in range(2)]
            ow_s = persist.tile([128, C], F32, tag="ows", name="ows")
            ow_r = persist.tile([128, C], F32R, tag="owr", name="owr")
            bsel_s = persist.tile([128, 128], F32, tag="bsels", name="bsels")
            bsel_r = persist.tile([128, 128], F32R, tag="bselr", name="bselr")
            gsel_t = [persist.tile([128, 128], F32, tag=f"gsel{i}", name=f"gsel{i}") for i in range(2)]
            gselT_t = persist.tile([128, C], F32, tag="gselT", name="gselT")
            nw_t = [persist.tile([128, 1], F32, tag=f"nw{i}", name=f"nw{i}") for i in range(2)]
            nb_t = [persist.tile([128, 1], F32, tag=f"nb{i}", name=f"nb{i}") for i in range(2)]
            ob_t = [persist.tile([128, 1], F32, tag=f"ob{i}", name=f"ob{i}") for i in range(2)]
            ones_h = persist.tile([128, 1], FP16, tag="ones", name="ones")
            eps_t = persist.tile([128, 1], F32, tag="eps", name="eps")
            esh_t = persist.tile([128, 1], F32, tag="esh", name="esh")
            ident_t = persist.tile([128, 128], FP16, tag="ident", name="ident")
            warm_t = persist.tile([128, 512], FP16, tag="warm", name="warm")
            nc.vector.memset(ones_h, 1.0)
            nc.vector.memset(eps_t, EPS)
            nc.vector.memset(esh_t, ESHIFT)
            nc.vector.memset(warm_t, 0.0)

            # x_kv gates the GroupNorm stats: per-queue DMA throughput is the
            # limit (~150GB/s each), so split it over all three DMA queues.
            # Weights + the first x_q chunks ride the scalar queue; the x_q
            # tail follows x_kv on sync/gpsimd (not needed until much later).
            xkv = [persist.tile([128, SEQ], BF16, tag=f"xkv{i}", name=f"xkv{i}") for i in range(2)]
            xq = [persist.tile([128, HALF], F32R, tag=f"xq{i}", name=f"xq{i}") for i in range(2)]
            for p in range(7):
                for i, q in ((0, nc.sync), (1, nc.gpsimd)):
                    q.dma_start(
                        out=xkv[i][:, 512 * p:512 * (p + 1)],
                        in_=x_kv[128 * i:128 * (i + 1), 512 * p:512 * (p + 1)],
                    )
            nc.scalar.dma_start(out=ow_s, in_=owT[:, :])
            nc.scalar.dma_start(out=bsel_s, in_=bsel[:, :])
            nc.scalar.dma_start(out=ident_t, in_=ident[:, :])
            nc.scalar.dma_start(out=gselT_t, in_=gselT[:, :])
            for i in range(2):
                nc.scalar.dma_start(out=wq_s[i], in_=wqkvT[128 * i:128 * (i + 1), :])
                nc.scalar.dma_start(out=gsel_t[i], in_=gsel[128 * i:128 * (i + 1), :])
                nc.scalar.dma_start(out=nw_t[i], in_=nw[128 * i:128 * (i + 1), :])
                nc.scalar.dma_start(out=nb_t[i], in_=nb[128 * i:128 * (i + 1), :])
                nc.scalar.dma_start(out=ob_t[i], in_=ob[128 * i:128 * (i + 1), :])
                nc.vector.tensor_copy(w_r[i], wq_s[i])
            for i in range(2):
                nc.scalar.dma_start(
                    out=xkv[i][:, 512 * 7:512 * 8],
                    in_=x_kv[128 * i:128 * (i + 1), 512 * 7:512 * 8],
                )
            for i in range(2):
                nc.scalar.dma_start(
                    out=xq[i][:, 0:512],
                    in_=x_q[128 * i:128 * (i + 1), 0:512],
                )
            for p in range(1, 4):
                for i, q in ((0, nc.sync), (1, nc.gpsimd)):
                    q.dma_start(
                        out=xq[i][:, 512 * p:512 * (p + 1)],
                        in_=x_q[128 * i:128 * (i + 1), 512 * p:512 * (p + 1)],
                    )
            nc.vector.tensor_copy(ow_r, ow_s)
            nc.vector.tensor_copy(bsel_r, bsel_s)

            # ---------------- GroupNorm statistics ----------------
            with tc.tile_pool(name="gn", bufs=1) as gn, \
                 tc.tile_pool(name="ps", bufs=2, space="PSUM") as ps:
                # preload the sqrt/exp ACT tables off the critical path
                scrA = gn.tile([128, 1], F32, tag="scrA", name="scrA")
                nc.scalar.activation(out=scrA, in_=eps_t, func=AF.Sqrt, bias=eps_t, scale=1.0)
                # dummy matmuls keep the PE out of its low p-state while the
                # x DMA + stats gate the real work
                dps = ps.tile([128, 2048], F32, tag="ps", name="ps")
                for w in range(16):
                    nc.tensor.matmul(dps[0:1, 512 * (w % 2):512 * (w % 2 + 1)],
                                     ones_h, warm_t, start=True, stop=True,
                                     skip_group_check=True)
                pp = [gn.tile([128, 2], F32, tag=f"pp{i}", name=f"pp{i}") for i in range(2)]
                for i in range(2):
                    stats = gn.tile([128, 8, 6], F32, tag=f"st{i}", name=f"st{i}")
                    for s in range(8):
                        nc.vector.bn_stats(out=stats[:, s, :], in_=xkv[i][:, 512 * s:512 * (s + 1)])
                    mv = gn.tile([128, 2], F32, tag=f"mv{i}", name=f"mv{i}")
                    nc.vector.bn_aggr(out=mv, in_=stats)
                    # pp = (mean, E[x^2]) per partition
                    tmp = gn.tile([128, 1], F32, tag=f"tmp{i}", name=f"tmp{i}")
                    nc.vector.tensor_copy(pp[i][:, 0:1], mv[:, 0:1])
                    nc.vector.tensor_mul(tmp, mv[:, 0:1], mv[:, 0:1])
                    nc.vector.tensor_add(pp[i][:, 1:2], mv[:, 1:2], tmp)

                # group sums: psum[g, :] = sum over channels of group g
                gs_ps = ps.tile([128, 2048], F32, tag="ps", name="ps")
                for i in range(2):
                    nc.tensor.matmul(gs_ps[:, 0:2], gsel_t[i], pp[i],
                                     start=(i == 0), stop=(i == 1))
                gsb = gn.tile([128, 2], F32, tag="gsb", name="gsb")
                # per-partition stats are already means over SEQ -> group mean = sum/32
                nc.vector.tensor_scalar_mul(gsb, gs_ps[:, 0:2], 1.0 / 32.0)
                gstats = gn.tile([128, 2], F32, tag="gstats", name="gstats")
                tmp2 = gn.tile([128, 1], F32, tag="tmp2", name="tmp2")
                varg = gn.tile([128, 1], F32, tag="varg", name="varg")
                nc.vector.tensor_copy(gstats[:, 0:1], gsb[:, 0:1])
                nc.vector.tensor_mul(tmp2, gsb[:, 0:1], gsb[:, 0:1])
                nc.vector.tensor_sub(varg, gsb[:, 1:2], tmp2)
                nc.scalar.activation(out=varg, in_=varg, func=AF.Sqrt, bias=eps_t, scale=1.0)
                # exp-table preload reads varg so the scheduler cannot hoist
                # it above the last Sqrt (which would evict the Exp table)
                nc.scalar.activation(out=scrA, in_=varg, func=AF.Exp)
                nc.vector.reciprocal(gstats[:, 1:2], varg)

                # broadcast group stats back to channels: cs[c] = (mean, rstd)
                cs = [gn.tile([128, 2], F32, tag=f"cs{i}", name=f"cs{i}") for i in range(2)]
                a_t = [gn.tile([128, 1], F32, tag=f"a{i}", name=f"a{i}") for i in range(2)]
                b_t = [gn.tile([128, 1], F32, tag=f"b{i}", name=f"b{i}") for i in range(2)]
                for i in range(2):
                    cs_ps = ps.tile([128, 2048], F32, tag="ps", name="ps")
                    nc.tensor.matmul(cs_ps[:, 0:2], gselT_t[:, 128 * i:128 * (i + 1)],
                                     gstats, start=True, stop=True)
                    nc.vector.tensor_copy(cs[i], cs_ps[:, 0:2])
                    tmp3 = gn.tile([128, 1], F32, tag=f"tmp3{i}", name=f"tmp3{i}")
                    nc.vector.tensor_mul(a_t[i], cs[i][:, 1:2], nw_t[i])
                    nc.vector.tensor_mul(tmp3, cs[i][:, 0:1], a_t[i])
                    nc.vector.tensor_sub(b_t[i], nb_t[i], tmp3)

                # ------------- QKV with GroupNorm folded into weights -------------
                # xn = a*x + b  =>  q = (Wq . a^T) x + Wq b  etc.  The V bias
                # passes through softmax as a constant (+vb after normalize).
                kq = persist.tile([128, SEQ], BF16, tag="K", name="K")
                qq = persist.tile([128, HALF], BF16, tag="Q", name="Q")
                vt_b = persist.tile([128, SEQ], FP16, tag="VT", name="VT")
                v_sb = persist.tile([128, SEQ], FP16, tag="Vsb", name="Vsb")
                w2_s = [persist.tile([128, 3 * HID], F32, tag=f"w2s{i}", name=f"w2s{i}") for i in range(2)]
                w2_r = [persist.tile([128, 3 * HID], F32R, tag=f"w2r{i}", name=f"w2r{i}") for i in range(2)]
                w2b = [persist.tile([128, 2 * HID], BF16, tag=f"w2b{i}", name=f"w2b{i}") for i in range(2)]
                qkvb = [persist.tile([128, 1], F32, tag=f"qkvb{m}", name=f"qkvb{m}") for m in range(3)]

                for i in range(2):
                    nc.vector.tensor_scalar_mul(w2_s[i], w_r[i].bitcast(F32), a_t[i])
                    nc.vector.tensor_copy(w2_r[i], w2_s[i])
                    # bf16 K/V weight copy -- the K/V projections stream the
                    # bf16 x_kv, so their weights must be bf16 too
                    nc.vector.tensor_copy(w2b[i], w2_s[i][:, HID:3 * HID])
                for m in range(3):
                    bp = ps.tile([128, 2048], F32, tag="ps", name="ps")
                    for i in range(2):
                        nc.tensor.matmul(bp[:, 0:1], wq_s[i][:, 128 * m:128 * (m + 1)],
                                         b_t[i], start=(i == 0), stop=(i == 1))
                    nc.vector.tensor_copy(qkvb[m], bp[:, 0:1])

                # only chunk 0's queries are needed to start the pipeline;
                # chunks 1-3 are projected from inside the slot loop
                qp = ps.tile([128, 2048], F32, tag="ps", name="ps")
                for i in range(2):
                    nc.tensor.matmul(qp[:, 0:512], w2_r[i][:, 0:HID],
                                     xq[i][:, 0:512],
                                     start=(i == 0), stop=(i == 1))
                nc.vector.tensor_scalar_add(qq[:, 0:512], qp[:, 0:512], qkvb[0])

            # ---------------- attention (v10) ----------------
            # 256 half-slots (c, t, p): S^T half-tile [128, 1024] (2 PSUM
            # banks, double-buffered) -> exp (ScalarE, fp16 out, shifted)
            # -> 2 PV matmuls into o_acc + 1 DVE fp16 add into Dp.
            with (
                tc.tile_pool(name="sgp", bufs=2, space="PSUM") as sgp,
                tc.tile_pool(name="accp", bufs=2, space="PSUM") as accp,
                tc.tile_pool(name="finp", bufs=2, space="PSUM") as finp,
                tc.tile_pool(name="apool", bufs=3) as apool,
                tc.tile_pool(name="fin", bufs=2) as fin,
            ):
                zrow = persist.tile([1, 512], FP16, tag="zrow", name="zrow")
                zcol = persist.tile([1, 128], FP16, tag="zcol", name="zcol")
                nc.vector.memset(zrow, 0.0)
                nc.vector.memset(zcol, 0.0)

                # fp16 denominator accumulators, double-buffered per chunk
                dp = [persist.tile([128, HALF], FP16, tag=f"dp{i}", name=f"dp{i}") for i in range(2)]
                nc.gpsimd.memset(dp[0], 0.0)
                nc.gpsimd.memset(dp[1], 0.0)

                slots = [(c, t, p) for c in range(N_IC) for t in range(N_JT)
                         for p in range(2)]
                sg_of = {}
                acc_of = {}

                def emit_S(idx):
                    c, t, p = slots[idx]
                    sg = sgp.tile([128, 1024], F32, tag="sg", name="sg")
                    for hh in range(2):
                        h = 2 * p + hh
                        nc.tensor.matmul(
                            sg[:, 512 * hh:512 * (hh + 1)],
                            kq[32 * h:32 * (h + 1), 128 * t:128 * (t + 1)],
                            qq[32 * h:32 * (h + 1), 512 * c:512 * (c + 1)],
                            start=True, stop=True, tile_position=(32 * h, 0),
                        )
                    sg_of[idx] = sg

                def emit_qproj(icb):
                    qp = finp.tile([128, 512], F32, tag="fp", name="qp")
                    for i in range(2):
                        nc.tensor.matmul(qp, w2_r[i][:, 0:HID],
                                         xq[i][:, 512 * icb:512 * (icb + 1)],
                                         start=(i == 0), stop=(i == 1))
                    nc.vector.tensor_scalar_add(qq[:, 512 * icb:512 * (icb + 1)],
                                                qp, qkvb[0])

                def emit_seg(seg):
                    sl = slice(512 * seg, 512 * (seg + 1))
                    kp = finp.tile([128, 512], F32, tag="fp", name="kp")
                    for i in range(2):
                        nc.tensor.matmul(kp, w2b[i][:, 0:HID],
                                         xkv[i][:, sl], start=(i == 0), stop=(i == 1))
                    nc.vector.tensor_scalar_add(kq[:, sl], kp, qkvb[1])
                    vp = finp.tile([128, 512], F32, tag="fp", name="vp")
                    for i in range(2):
                        nc.tensor.matmul(vp, w2b[i][:, HID:2 * HID],
                                         xkv[i][:, sl], start=(i == 0), stop=(i == 1))
                    nc.vector.tensor_copy(v_sb[:, sl], vp)
                    for tt in range(4):
                        t = 4 * seg + tt
                        tp = finp.tile([128, 256], FP16, tag="fp", name="tp")
                        nc.tensor.transpose(tp[:, 0:128],
                                            v_sb[:, 128 * t:128 * (t + 1)], ident_t)
                        nc.vector.tensor_copy(vt_b[:, 128 * t:128 * (t + 1)],
                                              tp[:, 0:128])

                def finalize(c, o_acc):
                    dcur = dp[c % 2]
                    # denominator: zero psum bank, then per-head ones-matmul
                    d4 = finp.tile([128, 512], F32, tag="fp", name="d4")
                    nc.tensor.matmul(d4, zcol, zrow, start=True, stop=False,
                                     skip_group_check=True)
                    for h in range(NH):
                        nc.tensor.matmul(
                            d4[32 * h:32 * h + 1, :], ones_h,
                            dcur[:, 512 * h:512 * (h + 1)],
                            start=False, stop=(h == NH - 1),
                            tile_position=(0, 32 * h), skip_group_check=True,
                        )
                    o_sb = fin.tile([128, 512], F32, tag="osb", name="osb")
                    d_sb = fin.tile([128, 512], F32, tag="dsb", name="dsb")
                    nc.vector.tensor_copy(o_sb, o_acc)
                    nc.vector.tensor_copy(d_sb, d4)
                    nc.vector.tensor_scalar_max(d_sb, d_sb, 1e-30)
                    dr32 = fin.tile([128, 512], F32, tag="dr32", name="dr32")
                    scr = fin.tile([128, 512], F32, tag="scr", name="scr")
                    dr = fin.tile([128, 512], F32R, tag="dr", name="dr")
                    nc.vector.reciprocal_approx_accurate(out=dr32, in_=d_sb,
                                                         scratch=scr)
                    nc.vector.tensor_copy(dr, dr32)
                    fsg = finp.tile([128, 512], F32, tag="fp", name="fsg")
                    nc.tensor.matmul(fsg, bsel_r, dr, start=True, stop=True)
                    on32 = fin.tile([128, 512], F32, tag="on32", name="on32")
                    on = fin.tile([128, 512], F32R, tag="on", name="on")
                    nc.vector.tensor_mul(on32, o_sb, fsg)
                    nc.vector.tensor_scalar_add(on, on32, qkvb[2])
                    for oc in range(2):
                        fo = finp.tile([128, 512], F32, tag="fp", name="fo")
                        nc.tensor.matmul(fo, ow_r[:, 128 * oc:128 * (oc + 1)],
                                         on, start=True, stop=True)
                        ysb = fin.tile([128, 512], F32, tag="ysb", name="ysb")
                        nc.vector.scalar_tensor_tensor(
                            out=ysb, in0=fo, scalar=ob_t[oc],
                            in1=xq[oc].bitcast(F32)[:, 512 * c:512 * (c + 1)],
                            op0=OP.add, op1=OP.add,
                        )
                        nc.sync.dma_start(
                            out=y[128 * oc:128 * (oc + 1), 512 * c:512 * (c + 1)],
                            in_=ysb,
                        )

                def emit_PV(idx, a_t2):
                    c, t, p = slots[idx]
                    o_acc = acc_of[c]
                    last = (t == N_JT - 1 and p == 1)
                    for hh in range(2):
                        h = 2 * p + hh
                        nc.tensor.matmul(
                            o_acc[32 * h:32 * (h + 1), :],
                            vt_b[:, 128 * t + 32 * h:128 * t + 32 * (h + 1)],
                            a_t2[:, 512 * hh:512 * (hh + 1)],
                            start=False, stop=(last and hh == 1),
                            tile_position=(0, 32 * h), skip_group_check=True,
                        )
                    if last:
                        finalize(c, o_acc)

                emit_seg(0)
                emit_S(0)
                a_of = {}
                for idx, (c, t, p) in enumerate(slots):
                    if t == 0 and p == 0:
                        o_acc = accp.tile([128, 512], F32, tag="Oacc", name="Oacc")
                        acc_of[c] = o_acc
                        nc.tensor.matmul(o_acc, zcol, zrow, start=True,
                                         stop=False, skip_group_check=True)

                    sg = sg_of.pop(idx)
                    a_t2 = apool.tile([128, 1024], FP16, tag="A", name="A")
                    a_of[idx] = a_t2
                    nc.scalar.activation(out=a_t2, in_=sg, func=AF.Exp,
                                         scale=SCALE, bias=esh_t)
                    # S of the next half-slot goes on the PE queue BEFORE the
                    # delayed PV so that, when exp(idx) completes, the PE runs
                    # S(idx+1) first -- exp(idx+1)'s input is ready with a full
                    # exp-duration of margin and ScalarE never waits on the PE.
                    if idx + 1 < len(slots):
                        emit_S(idx + 1)
                    if idx > 0:
                        emit_PV(idx - 1, a_of.pop(idx - 1))
                    if t == 0 and p == 0 and c + 1 < N_IC:
                        # after the delayed PV/finalize of chunk c-1 so the
                        # memset's WAR lands behind finalize's dp reads
                        nc.gpsimd.memset(dp[(c + 1) % 2], 0.0)
                    nc.vector.tensor_add(dp[c % 2][:, 1024 * p:1024 * (p + 1)],
                                         dp[c % 2][:, 1024 * p:1024 * (p + 1)],
                                         a_t2)
                    if c == 0 and p == 1 and t in (1, 2, 3):
                        emit_qproj(t)
                    if c == 0 and p == 0 and t % 4 == 0 and t // 4 + 1 < 8:
                        emit_seg(t // 4 + 1)
                n_last = len(slots) - 1
                emit_PV(n_last, a_of.pop(n_last))
    nc.compile()
    return nc


_NC_CACHE = {}


def _get_nc():
    if "nc" not in _NC_CACHE:
        _NC_CACHE["nc"] = build_program()
    return _NC_CACHE["nc"]


def _host_inputs(x, norm_w, norm_b, qkv_w, out_w, out_b):
    """Build the 8 per-core input maps."""
    x = np.asarray(x, dtype=np.float32)
    B = x.shape[0]
    xf = x.reshape(B, C, SEQ)

    wqkvT = np.ascontiguousarray(np.asarray(qkv_w, np.float32).T)      # [256, 384]
    owT = np.ascontiguousarray(np.asarray(out_w, np.float32).T)        # [128, 256]
    nw = np.asarray(norm_w, np.float32).reshape(C, 1).copy()
    nb = np.asarray(norm_b, np.float32).reshape(C, 1).copy()
    ob = np.asarray(out_b, np.float32).reshape(C, 1).copy()

    gsel = np.zeros((C, 128), np.float32)
    for ch in range(C):
        gsel[ch, ch // 32] = 1.0
    gselT = np.ascontiguousarray(gsel.T)                               # [128, 256]
    bsel = np.zeros((128, 128), np.float32)
    for m in range(128):
        bsel[32 * (m // 32), m] = 1.0
    ident = np.eye(128, dtype=np.float16)

    import ml_dtypes
    in_maps = []
    for core in range(8):
        b, h = core // 2, core % 2
        in_maps.append({
            "x_kv": np.ascontiguousarray(xf[b].astype(ml_dtypes.bfloat16)),
            "x_q": np.ascontiguousarray(xf[b][:, HALF * h:HALF * (h + 1)]),
            "wqkvT": wqkvT, "owT": owT, "nw": nw, "nb": nb, "ob": ob,
            "gsel": gsel, "gselT": gselT, "bsel": bsel, "ident": ident,
        })
    return in_maps


def run(x, norm_w, norm_b, qkv_w, out_w, out_b, trace=False, tmpdir=None):
    """Run on 8 cores; returns (y_full, BassKernelResults)."""
    nc = _get_nc()
    in_maps = _host_inputs(x, norm_w, norm_b, qkv_w, out_w, out_b)
    res = run_bass_kernel_spmd(nc, in_maps, core_ids=list(range(8)), trace=trace,
                               tmpdir=tmpdir)
    B = np.asarray(x).shape[0]
    HW_SIDE = int(np.sqrt(SEQ))
    out = np.empty((B, C, SEQ), np.float32)
    for core in range(8):
        b, h = core // 2, core % 2
        out[b][:, HALF * h:HALF * (h + 1)] = res.results[core]["y"]
    return out.reshape(B, C, HW_SIDE, HW_SIDE), res


def kernel(x, norm_w, norm_b, qkv_w, out_w, out_b):
    y, _ = run(x, norm_w, norm_b, qkv_w, out_w, out_b, trace=False)
    return y


# revision 37
# speedup vs baseline: 1.1972x; 1.1972x over previous
"""Trainium2 Bass kernel for the SD-style spatial attention block:

    y = x + out_w @ attn(qkv(groupnorm(x))) + out_b    (per sample)

x: [4, 256, 64, 64] fp32.  GroupNorm(8 groups) -> 1x1 conv QKV (4 heads,
head_dim 32, seq = 64*64 = 4096) -> softmax attention -> 1x1 out conv + bias
+ residual.

Sharding over 8 NeuronCores: core c handles batch b = c//2 and query-half
h = c%2 (2048 of the 4096 query positions).  Each core receives the full
sample (for GroupNorm stats and K/V over all positions) plus its query
slice, and produces the disjoint output slice y[b][:, 2048*h : 2048*(h+1)].
The host concatenates the 8 slices -- no cross-core reduction.

v10 pipeline (per core), designed to make ScalarE (exp) the only
bottleneck and keep it gapless:
  - attention runs in 256 half-slots (chunk c of 512 queries, j-tile t of
    128 keys, head-pair p in {01, 23}).  S^T half-tiles [128, 1024] live
    in a double-buffered 2-bank PSUM pool, so the next half-slot's S
    matmuls (PE) overlap the current exp (ScalarE).
  - softmax denominators come from DVE fp16 adds (4x perf mode) of the
    exp output A into a per-chunk [128, 2048] fp16 accumulator, then one
    ones-matmul per head per chunk - the per-slot ones-matmuls that used
    to eat a third of the PE are gone.
  - exp is computed as exp(S*scale - 2); the constant shift keeps A and
    the fp16 partial sums in fp16 range and cancels in O/D.
  - QKV/out projections consume x directly as f32r (no bf16 cast pass).
"""
import sys

sys.path.insert(0, "/opt/trn_rl_repo")

import numpy as np

import concourse.bass as bass
import concourse.bacc as bacc
import concourse.tile as tile
from concourse import mybir
from concourse.bass_utils import run_bass_kernel_spmd

F32 = mybir.dt.float32
F32R = mybir.dt.float32r
BF16 = mybir.dt.bfloat16
FP16 = mybir.dt.float16
AF = mybir.ActivationFunctionType
OP = mybir.AluOpType

C = 256          # input channels
HID = 128        # qkv hidden (4 heads x 32)
NH = 4
HD = 32
SEQ = 4096       # 64*64 spatial positions
HALF = 2048      # query positions per core
G = 8            # groups
EPS = 1e-5
SCALE = float(HD) ** -0.5
ESHIFT = -2.0    # constant exp shift; cancels in O/D normalization

N_IC = HALF // 512   # i-chunks per core (4)
N_JT = SEQ // 128    # j-tiles (32)


def build_program():
    nc = bacc.Bacc()

    x_kv = nc.declare_dram_parameter("x_kv", [C, SEQ], BF16, isOutput=False)
    x_q = nc.declare_dram_parameter("x_q", [C, HALF], F32R, isOutput=False)
    wqkvT = nc.declare_dram_parameter("wqkvT", [C, 3 * HID], F32, isOutput=False)
    owT = nc.declare_dram_parameter("owT", [HID, C], F32, isOutput=False)
    nw = nc.declare_dram_parameter("nw", [C, 1], F32, isOutput=False)
    nb = nc.declare_dram_parameter("nb", [C, 1], F32, isOutput=False)
    ob = nc.declare_dram_parameter("ob", [C, 1], F32, isOutput=False)
    gsel = nc.declare_dram_parameter("gsel", [C, 128], F32, isOutput=False)
    gselT = nc.declare_dram_parameter("gselT", [128, C], F32, isOutput=False)
    bsel = nc.declare_dram_parameter("bsel", [128, 128], F32, isOutput=False)
    ident = nc.declare_dram_parameter("ident", [128, 128], FP16, isOutput=False)
    y = nc.declare_dram_parameter("y", [C, HALF], F32, isOutput=True)

    with tile.TileContext(nc) as tc:
        import contextlib
        with contextlib.ExitStack() as ctx:
            persist = ctx.enter_context(tc.tile_pool(name="persist", bufs=1))

            # ---------------- load persistent tensors ----------------
            # weights staged in fp32, laundered to f32r via DVE copies
            wq_s = [persist.tile([128, 3 * HID], F32, tag=f"wqs{i}", name=f"wqs{i}") for i in range(2)]
            w_r = [persist.tile([128, 3 * HID], F32R, tag=f"wqr{i}", name=f"wqr{i}") for i in range(2)]
            ow_s = persist.tile([128, C], F32, tag="ows", name="ows")
            ow_r = persist.tile([128, C], F32R, tag="owr", name="owr")
            bsel_s = persist.tile([128, 128], F32, tag="bsels", name="bsels")
            bsel_r = persist.tile([128, 128], F32R, tag="bselr", name="bselr")
            gsel_t = [persist.tile([128, 128], F32, tag=f"gsel{i}", name=f"gsel{i}") for i in range(2)]
            gselT_t = persist.tile([128, C], F32, tag="gselT", name="gselT")
            nw_t = [persist.tile([128, 1], F32, tag=f"nw{i}", name=f"nw{i}") for i in range(2)]
            nb_t = [persist.tile([128, 1], F32, tag=f"nb{i}", name=f"nb{i}") for i in range(2)]
            ob_t = [persist.tile([128, 1], F32, tag=f"ob{i}", name=f"ob{i}") for i in range(2)]
            ones_h = persist.tile([128, 1], FP16, tag="ones", name="ones")
            eps_t = persist.tile([128, 1], F32, tag="eps", name="eps")
            esh_t = persist.tile([128, 1], F32, tag="esh", name="esh")
            warm_t = persist.tile([128, 512], FP16, tag="warm", name="warm")
            nc.vector.memset(ones_h, 1.0)
            nc.vector.memset(eps_t, EPS)
            nc.vector.memset(esh_t, ESHIFT)
            nc.vector.memset(warm_t, 0.0)

            # x_kv gates the GroupNorm stats: per-queue DMA throughput is the
            # limit (~150GB/s each), so split it over all three DMA queues.
            # Weights + the first x_q chunks ride the scalar queue; the x_q
            # tail follows x_kv on sync/gpsimd (not needed until much later).
            xkv = [persist.tile([128, SEQ], BF16, tag=f"xkv{i}", name=f"xkv{i}") for i in range(2)]
            xq = [persist.tile([128, HALF], F32R, tag=f"xq{i}", name=f"xq{i}") for i in range(2)]
            for p in range(7):
                for i, q in ((0, nc.sync), (1, nc.gpsimd)):
                    q.dma_start(
                        out=xkv[i][:, 512 * p:512 * (p + 1)],
                        in_=x_kv[128 * i:128 * (i + 1), 512 * p:512 * (p + 1)],
                    )
            nc.scalar.dma_start(out=ow_s, in_=owT[:, :])
            nc.scalar.dma_start(out=bsel_s, in_=bsel[:, :])
            nc.scalar.dma_start(out=gselT_t, in_=gselT[:, :])
            for i in range(2):
                nc.scalar.dma_start(out=wq_s[i], in_=wqkvT[128 * i:128 * (i + 1), :])
                nc.scalar.dma_start(out=gsel_t[i], in_=gsel[128 * i:128 * (i + 1), :])
                nc.scalar.dma_start(out=nw_t[i], in_=nw[128 * i:128 * (i + 1), :])
                nc.scalar.dma_start(out=nb_t[i], in_=nb[128 * i:128 * (i + 1), :])
                nc.scalar.dma_start(out=ob_t[i], in_=ob[128 * i:128 * (i + 1), :])
                nc.vector.tensor_copy(w_r[i], wq_s[i])
            for i in range(2):
                nc.scalar.dma_start(
                    out=xkv[i][:, 512 * 7:512 * 8],
                    in_=x_kv[128 * i:128 * (i + 1), 512 * 7:512 * 8],
                )
            for i in range(2):
                nc.scalar.dma_start(
                    out=xq[i][:, 0:512],
                    in_=x_q[128 * i:128 * (i + 1), 0:512],
                )
            for p in range(1, 4):
                for i, q in ((0, nc.sync), (1, nc.gpsimd)):
                    q.dma_start(
                        out=xq[i][:, 512 * p:512 * (p + 1)],
                        in_=x_q[128 * i:128 * (i + 1), 512 * p:512 * (p + 1)],
                    )
            nc.vector.tensor_copy(ow_r, ow_s)
            nc.vector.tensor_copy(bsel_r, bsel_s)

            # ---------------- GroupNorm statistics ----------------
            with tc.tile_pool(name="gn", bufs=1) as gn, \
                 tc.tile_pool(name="ps", bufs=2, space="PSUM") as ps:
                # preload the sqrt/exp ACT tables off the critical path
                scrA = gn.tile([128, 1], F32, tag="scrA", name="scrA")
                nc.scalar.activation(out=scrA, in_=eps_t, func=AF.Sqrt, bias=eps_t, scale=1.0)
                # dummy matmuls keep the PE out of its low p-state while the
                # x DMA + stats gate the real work
                dps = ps.tile([128, 2048], F32, tag="ps", name="ps")
                for w in range(16):
                    nc.tensor.matmul(dps[0:1, 512 * (w % 2):512 * (w % 2 + 1)],
                                     ones_h, warm_t, start=True, stop=True,
                                     skip_group_check=True)
                pp = [gn.tile([128, 2], F32, tag=f"pp{i}", name=f"pp{i}") for i in range(2)]
                for i in range(2):
                    stats = gn.tile([128, 8, 6], F32, tag=f"st{i}", name=f"st{i}")
                    for s in range(8):
                        nc.vector.bn_stats(out=stats[:, s, :], in_=xkv[i][:, 512 * s:512 * (s + 1)])
                    mv = gn.tile([128, 2], F32, tag=f"mv{i}", name=f"mv{i}")
                    nc.vector.bn_aggr(out=mv, in_=stats)
                    # pp = (mean, E[x^2]) per partition
                    tmp = gn.tile([128, 1], F32, tag=f"tmp{i}", name=f"tmp{i}")
                    nc.vector.tensor_copy(pp[i][:, 0:1], mv[:, 0:1])
                    nc.vector.tensor_mul(tmp, mv[:, 0:1], mv[:, 0:1])
                    nc.vector.tensor_add(pp[i][:, 1:2], mv[:, 1:2], tmp)

                # group sums: psum[g, :] = sum over channels of group g
                gs_ps = ps.tile([128, 2048], F32, tag="ps", name="ps")
                for i in range(2):
                    nc.tensor.matmul(gs_ps[:, 0:2], gsel_t[i], pp[i],
                                     start=(i == 0), stop=(i == 1))
                gsb = gn.tile([128, 2], F32, tag="gsb", name="gsb")
                # per-partition stats are already means over SEQ -> group mean = sum/32
                nc.vector.tensor_scalar_mul(gsb, gs_ps[:, 0:2], 1.0 / 32.0)
                gstats = gn.tile([128, 2], F32, tag="gstats", name="gstats")
                tmp2 = gn.tile([128, 1], F32, tag="tmp2", name="tmp2")
                varg = gn.tile([128, 1], F32, tag="varg", name="varg")
                nc.vector.tensor_copy(gstats[:, 0:1], gsb[:, 0:1])
                nc.vector.tensor_mul(tmp2, gsb[:, 0:1], gsb[:, 0:1])
                nc.vector.tensor_sub(varg, gsb[:, 1:2], tmp2)
                nc.scalar.activation(out=varg, in_=varg, func=AF.Sqrt, bias=eps_t, scale=1.0)
                # exp-table preload reads varg so the scheduler cannot hoist
                # it above the last Sqrt (which would evict the Exp table)
                nc.scalar.activation(out=scrA, in_=varg, func=AF.Exp)
                nc.vector.reciprocal(gstats[:, 1:2], varg)

                # broadcast group stats back to channels: cs[c] = (mean, rstd)
                cs = [gn.tile([128, 2], F32, tag=f"cs{i}", name=f"cs{i}") for i in range(2)]
                a_t = [gn.tile([128, 1], F32, tag=f"a{i}", name=f"a{i}") for i in range(2)]
                b_t = [gn.tile([128, 1], F32, tag=f"b{i}", name=f"b{i}") for i in range(2)]
                for i in range(2):
                    cs_ps = ps.tile([128, 2048], F32, tag="ps", name="ps")
                    nc.tensor.matmul(cs_ps[:, 0:2], gselT_t[:, 128 * i:128 * (i + 1)],
                                     gstats, start=True, stop=True)
                    nc.vector.tensor_copy(cs[i], cs_ps[:, 0:2])
                    tmp3 = gn.tile([128, 1], F32, tag=f"tmp3{i}", name=f"tmp3{i}")
                    nc.vector.tensor_mul(a_t[i], cs[i][:, 1:2], nw_t[i])
                    nc.vector.tensor_mul(tmp3, cs[i][:, 0:1], a_t[i])
                    nc.vector.tensor_sub(b_t[i], nb_t[i], tmp3)

                # ------------- QKV with GroupNorm folded into weights -------------
                # xn = a*x + b  =>  q = (Wq . a^T) x + Wq b  etc.  The V bias
                # passes through softmax as a constant (+vb after normalize).
                kq = persist.tile([128, SEQ], BF16, tag="K", name="K")
                qq = persist.tile([128, HALF], BF16, tag="Q", name="Q")
                vt_b = persist.tile([128, SEQ], FP16, tag="VT", name="VT")
                w2_s = [persist.tile([128, 3 * HID], F32, tag=f"w2s{i}", name=f"w2s{i}") for i in range(2)]
                w2_r = [persist.tile([128, 3 * HID], F32R, tag=f"w2r{i}", name=f"w2r{i}") for i in range(2)]
                w2b = [persist.tile([128, 2 * HID], BF16, tag=f"w2b{i}", name=f"w2b{i}") for i in range(2)]
                qkvb = [persist.tile([128, 1], F32, tag=f"qkvb{m}", name=f"qkvb{m}") for m in range(3)]

                for i in range(2):
                    nc.vector.tensor_scalar_mul(w2_s[i], w_r[i].bitcast(F32), a_t[i])
                    nc.vector.tensor_copy(w2_r[i], w2_s[i])
                    # bf16 K/V weight copy -- the K/V projections stream the
                    # bf16 x_kv, so their weights must be bf16 too
                    nc.vector.tensor_copy(w2b[i], w2_s[i][:, HID:3 * HID])
                for m in range(3):
                    bp = ps.tile([128, 2048], F32, tag="ps", name="ps")
                    for i in range(2):
                        nc.tensor.matmul(bp[:, 0:1], wq_s[i][:, 128 * m:128 * (m + 1)],
                                         b_t[i], start=(i == 0), stop=(i == 1))
                    nc.vector.tensor_copy(qkvb[m], bp[:, 0:1])

                # only chunk 0's queries are needed to start the pipeline;
                # chunks 1-3 are projected from inside the slot loop
                qp = ps.tile([128, 2048], F32, tag="ps", name="ps")
                for i in range(2):
                    nc.tensor.matmul(qp[:, 0:512], w2_r[i][:, 0:HID],
                                     xq[i][:, 0:512],
                                     start=(i == 0), stop=(i == 1))
                nc.vector.tensor_scalar_add(qq[:, 0:512], qp[:, 0:512], qkvb[0])

            # ---------------- attention (v10) ----------------
            # 256 half-slots (c, t, p): S^T half-tile [128, 1024] (2 PSUM
            # banks, double-buffered) -> exp (ScalarE, fp16 out, shifted)
            # -> 2 PV matmuls into o_acc + 1 DVE fp16 add into Dp.
            with (
                tc.tile_pool(name="sgp", bufs=2, space="PSUM") as sgp,
                tc.tile_pool(name="accp", bufs=2, space="PSUM") as accp,
                tc.tile_pool(name="finp", bufs=2, space="PSUM") as finp,
                tc.tile_pool(name="apool", bufs=3) as apool,
                tc.tile_pool(name="fin", bufs=2) as fin,
            ):
                zrow = persist.tile([1, 512], FP16, tag="zrow", name="zrow")
                zcol = persist.tile([1, 128], FP16, tag="zcol", name="zcol")
                nc.vector.memset(zrow, 0.0)
                nc.vector.memset(zcol, 0.0)

                # fp16 denominator accumulators, double-buffered per chunk
                dp = [persist.tile([128, HALF], FP16, tag=f"dp{i}", name=f"dp{i}") for i in range(2)]
                nc.gpsimd.memset(dp[0], 0.0)
                nc.gpsimd.memset(dp[1], 0.0)

                slots = [(c, t, p) for c in range(N_IC) for t in range(N_JT)
                         for p in range(2)]
                sg_of = {}
                acc_of = {}

                def emit_S(idx):
                    c, t, p = slots[idx]
                    sg = sgp.tile([128, 1024], F32, tag="sg", name="sg")
                    for hh in range(2):
                        h = 2 * p + hh
                        nc.tensor.matmul(
                            sg[:, 512 * hh:512 * (hh + 1)],
                            kq[32 * h:32 * (h + 1), 128 * t:128 * (t + 1)],
                            qq[32 * h:32 * (h + 1), 512 * c:512 * (c + 1)],
                            start=True, stop=True, tile_position=(32 * h, 0),
                        )
                    sg_of[idx] = sg

                def emit_qproj(icb):
                    qp = finp.tile([128, 512], F32, tag="fp", name="qp")
                    for i in range(2):
                        nc.tensor.matmul(qp, w2_r[i][:, 0:HID],
                                         xq[i][:, 512 * icb:512 * (icb + 1)],
                                         start=(i == 0), stop=(i == 1))
                    nc.vector.tensor_scalar_add(qq[:, 512 * icb:512 * (icb + 1)],
                                                qp, qkvb[0])

                def emit_seg(seg):
                    sl = slice(512 * seg, 512 * (seg + 1))
                    kp = finp.tile([128, 512], F32, tag="fp", name="kp")
                    for i in range(2):
                        nc.tensor.matmul(kp, w2b[i][:, 0:HID],
                                         xkv[i][:, sl], start=(i == 0), stop=(i == 1))
                    nc.vector.tensor_scalar_add(kq[:, sl], kp, qkvb[1])
                    # V^T produced directly: out[key, (h,d)] = x^T . (a*Wv)^T,
                    # swapping matmul operand roles - no PE transposes needed
                    for tt in range(4):
                        t = 4 * seg + tt
                        vtp = finp.tile([128, 128], F32, tag="fp", name="vtp")
                        for i in range(2):
                            nc.tensor.matmul(vtp, xkv[i][:, 128 * t:128 * (t + 1)],
                                             w2b[i][:, HID:2 * HID],
                                             start=(i == 0), stop=(i == 1))
                        nc.vector.tensor_copy(vt_b[:, 128 * t:128 * (t + 1)], vtp)

                def finalize(c, o_acc):
                    dcur = dp[c % 2]
                    # denominator: zero psum bank, then per-head ones-matmul
                    d4 = finp.tile([128, 512], F32, tag="fp", name="d4")
                    nc.tensor.matmul(d4, zcol, zrow, start=True, stop=False,
                                     skip_group_check=True)
                    for h in range(NH):
                        nc.tensor.matmul(
                            d4[32 * h:32 * h + 1, :], ones_h,
                            dcur[:, 512 * h:512 * (h + 1)],
                            start=False, stop=(h == NH - 1),
                            tile_position=(0, 32 * h), skip_group_check=True,
                        )
                    o_sb = fin.tile([128, 512], F32, tag="osb", name="osb")
                    d_sb = fin.tile([128, 512], F32, tag="dsb", name="dsb")
                    nc.vector.tensor_copy(o_sb, o_acc)
                    nc.vector.tensor_copy(d_sb, d4)
                    nc.vector.tensor_scalar_max(d_sb, d_sb, 1e-30)
                    dr32 = fin.tile([128, 512], F32, tag="dr32", name="dr32")
                    scr = fin.tile([128, 512], F32, tag="scr", name="scr")
                    dr = fin.tile([128, 512], F32R, tag="dr", name="dr")
                    nc.vector.reciprocal_approx_accurate(out=dr32, in_=d_sb,
                                                         scratch=scr)
                    nc.vector.tensor_copy(dr, dr32)
                    fsg = finp.tile([128, 512], F32, tag="fp", name="fsg")
                    nc.tensor.matmul(fsg, bsel_r, dr, start=True, stop=True)
                    on32 = fin.tile([128, 512], F32, tag="on32", name="on32")
                    on = fin.tile([128, 512], F32R, tag="on", name="on")
                    nc.vector.tensor_mul(on32, o_sb, fsg)
                    nc.vector.tensor_scalar_add(on, on32, qkvb[2])
                    for oc in range(2):
                        fo = finp.tile([128, 512], F32, tag="fp", name="fo")
                        nc.tensor.matmul(fo, ow_r[:, 128 * oc:128 * (oc + 1)],
                                         on, start=True, stop=True)
                        ysb = fin.tile([128, 512], F32, tag="ysb", name="ysb")
                        nc.vector.scalar_tensor_tensor(
                            out=ysb, in0=fo, scalar=ob_t[oc],
                            in1=xq[oc].bitcast(F32)[:, 512 * c:512 * (c + 1)],
                            op0=OP.add, op1=OP.add,
                        )
                        nc.sync.dma_start(
                            out=y[128 * oc:128 * (oc + 1), 512 * c:512 * (c + 1)],
                            in_=ysb,
                        )

                def emit_PV(idx, a_t2):
                    c, t, p = slots[idx]
                    o_acc = acc_of[c]
                    last = (t == N_JT - 1 and p == 1)
                    for hh in range(2):
                        h = 2 * p + hh
                        nc.tensor.matmul(
                            o_acc[32 * h:32 * (h + 1), :],
                            vt_b[:, 128 * t + 32 * h:128 * t + 32 * (h + 1)],
                            a_t2[:, 512 * hh:512 * (hh + 1)],
                            start=False, stop=(last and hh == 1),
                            tile_position=(0, 32 * h), skip_group_check=True,
                        )
                    if last:
                        finalize(c, o_acc)

                emit_seg(0)
                emit_S(0)
                a_of = {}
                for idx, (c, t, p) in enumerate(slots):
                    if t == 0 and p == 0:
                        o_acc = accp.tile([128, 512], F32, tag="Oacc", name="Oacc")
                        acc_of[c] = o_acc
                        nc.tensor.matmul(o_acc, zcol, zrow, start=True,
                                         stop=False, skip_group_check=True)

                    sg = sg_of.pop(idx)
                    a_t2 = apool.tile([128, 1024], FP16, tag="A", name="A")
                    a_of[idx] = a_t2
                    nc.scalar.activation(out=a_t2, in_=sg, func=AF.Exp,
                                         scale=SCALE, bias=esh_t)
                    # S of the next half-slot goes on the PE queue BEFORE the
                    # delayed PV so that, when exp(idx) completes, the PE runs
                    # S(idx+1) first -- exp(idx+1)'s input is ready with a full
                    # exp-duration of margin and ScalarE never waits on the PE.
                    if idx + 1 < len(slots):
                        emit_S(idx + 1)
                    if idx > 0:
                        emit_PV(idx - 1, a_of.pop(idx - 1))
                    if t == 0 and p == 0 and c + 1 < N_IC:
                        # after the delayed PV/finalize of chunk c-1 so the
                        # memset's WAR lands behind finalize's dp reads
                        nc.gpsimd.memset(dp[(c + 1) % 2], 0.0)
                    nc.vector.tensor_add(dp[c % 2][:, 1024 * p:1024 * (p + 1)],
                                         dp[c % 2][:, 1024 * p:1024 * (p + 1)],
                                         a_t2)
                    if c == 0 and p == 1 and t in (1, 2, 3):
                        emit_qproj(t)
                    if c == 0 and p == 0 and t % 4 == 0 and t // 4 + 1 < 8:
                        emit_seg(t // 4 + 1)
                n_last = len(slots) - 1
                emit_PV(n_last, a_of.pop(n_last))
    nc.compile()
    return nc


_NC_CACHE = {}


def _get_nc():
    if "nc" not in _NC_CACHE:
        _NC_CACHE["nc"] = build_program()
    return _NC_CACHE["nc"]


def _host_inputs(x, norm_w, norm_b, qkv_w, out_w, out_b):
    """Build the 8 per-core input maps."""
    x = np.asarray(x, dtype=np.float32)
    B = x.shape[0]
    xf = x.reshape(B, C, SEQ)

    wqkvT = np.ascontiguousarray(np.asarray(qkv_w, np.float32).T)      # [256, 384]
    owT = np.ascontiguousarray(np.asarray(out_w, np.float32).T)        # [128, 256]
    nw = np.asarray(norm_w, np.float32).reshape(C, 1).copy()
    nb = np.asarray(norm_b, np.float32).reshape(C, 1).copy()
    ob = np.asarray(out_b, np.float32).reshape(C, 1).copy()

    gsel = np.zeros((C, 128), np.float32)
    for ch in range(C):
        gsel[ch, ch // 32] = 1.0
    gselT = np.ascontiguousarray(gsel.T)                               # [128, 256]
    bsel = np.zeros((128, 128), np.float32)
    for m in range(128):
        bsel[32 * (m // 32), m] = 1.0
    ident = np.eye(128, dtype=np.float16)

    import ml_dtypes
    in_maps = []
    for core in range(8):
        b, h = core // 2, core % 2
        in_maps.append({
            "x_kv": np.ascontiguousarray(xf[b].astype(ml_dtypes.bfloat16)),
            "x_q": np.ascontiguousarray(xf[b][:, HALF * h:HALF * (h + 1)]),
            "wqkvT": wqkvT, "owT": owT, "nw": nw, "nb": nb, "ob": ob,
            "gsel": gsel, "gselT": gselT, "bsel": bsel, "ident": ident,
        })
    return in_maps


def run(x, norm_w, norm_b, qkv_w, out_w, out_b, trace=False, tmpdir=None):
    """Run on 8 cores; returns (y_full, BassKernelResults)."""
    nc = _get_nc()
    in_maps = _host_inputs(x, norm_w, norm_b, qkv_w, out_w, out_b)
    res = run_bass_kernel_spmd(nc, in_maps, core_ids=list(range(8)), trace=trace,
                               tmpdir=tmpdir)
    B = np.asarray(x).shape[0]
    HW_SIDE = int(np.sqrt(SEQ))
    out = np.empty((B, C, SEQ), np.float32)
    for core in range(8):
        b, h = core // 2, core % 2
        out[b][:, HALF * h:HALF * (h + 1)] = res.results[core]["y"]
    return out.reshape(B, C, HW_SIDE, HW_SIDE), res


def kernel(x, norm_w, norm_b, qkv_w, out_w, out_b):
    y, _ = run(x, norm_w, norm_b, qkv_w, out_w, out_b, trace=False)
    return y
